# revision 4
# baseline (speedup 1.0000x reference)
"""Trainium2 Bass kernel for KeyframeSelectionNetwork (v2).

Math (per (b, v) video of T=64 frames, F=1024 features):
  GCN with self-loops + one edge (frame0 -> frame1), symmetric norm:
    out[t] = x[t] @ W_gcn                      for t != 1
    out[1] = (0.5*x[1] + (1/sqrt(2))*x[0]) @ W_gcn
  pooled = max_t out[t] + b_gcn
  h = relu(pooled.reshape(B, V*F) @ W1 + b1)
  key = sigmoid(h @ W2 + b2)            -> [B, V, T]

Strategy: data-parallel over batch across 8 cores. The frame-0/1 combine is
elementwise on X and commutes with the matmul, so it is applied on the HOST,
as is the [node, F] -> [F, node] transpose (pure layout) and the downcast
(X/W_gcn to fp8 e4m3, MLP weights to bf16; fp32 accumulation throughout
keeps rel-err ~9e-3, well under the 2e-2 gate). The device runs one clean
GEMM pipeline per core:
  - per 1024-node group g: load XT tile [128, kk, 2, 1024] fp8 (one 1MB DMA);
  - for each 128-wide output block j: PSUM-accumulated Double-FP8 matmuls
    (256-row contraction per instruction: W_gcn block [p,2,128] stationary,
    XT [p,2,512] moving) into a 2-bank [128,1024] PSUM tile;
  - one 3D-AP reduce_max over t per (g, j) writes pooledT [fout, b*v];
  - MLP in bf16: pooledT slices stationary vs W1 [128,256] moving, biases as
    rank-1 matmuls (ones.T @ b), relu/sigmoid on ACT, h transposed via PE.
"""

import sys

sys.path.insert(0, "/opt/trn_rl_repo")

import numpy as np
import ml_dtypes

BF16 = ml_dtypes.bfloat16

B, V, T, F = 64, 8, 64, 1024
NCORES = 8
BL = B // NCORES  # batches per core
NLOC = BL * V * T  # nodes per core (4096)
H1 = 256
OUT = V * T  # 512
P = 128
GRP = 1024  # nodes per group (matmul moving-N)
NG = NLOC // GRP  # 4 groups
KC = F // P  # 8 contraction chunks
JC = F // P  # 8 output-feature chunks

CFG = dict(
    psum_bufs=3,
    x_bufs=3,
    gcn_fp8=True,  # e4m3 X/W_gcn + DoubleRow matmul (2x PE rate, halved X bytes)
)

_STATE = None


def _build_nc(cfg, reps=1):
    import concourse.bacc as bacc
    import concourse.tile as tile
    from concourse import mybir

    f32 = mybir.dt.float32
    bf16 = mybir.dt.bfloat16
    fp8 = mybir.dt.float8e4
    AF = mybir.ActivationFunctionType
    use_fp8 = cfg.get("gcn_fp8", False)
    g_dt = fp8 if use_fp8 else bf16

    nc = bacc.Bacc(None, target_bir_lowering=False, debug=False)
    xt_d = nc.dram_tensor("videosT", [F, NLOC], g_dt, kind="ExternalInput")
    wg_d = nc.dram_tensor("W_gcn", [F, F], g_dt, kind="ExternalInput")
    bg_d = nc.dram_tensor("b_gcn", [F], f32, kind="ExternalInput")
    w1_d = nc.dram_tensor("W1", [V * F, H1], bf16, kind="ExternalInput")
    b1_d = nc.dram_tensor("b1", [H1], f32, kind="ExternalInput")
    w2_d = nc.dram_tensor("W2", [H1, OUT], bf16, kind="ExternalInput")
    b2_d = nc.dram_tensor("b2", [OUT], f32, kind="ExternalInput")
    id8_d = nc.dram_tensor("id8", [BL, BL], f32, kind="ExternalInput")
    if reps == 1:
        out_d = nc.dram_tensor("out", [BL, OUT], f32, kind="ExternalOutput")
    else:
        # distinct per-rep outputs so DCE can't drop repeated workloads
        out_d = nc.dram_tensor("out", [reps, BL, OUT], f32, kind="ExternalOutput")

    with tile.TileContext(nc) as tc:
      with (
          tc.tile_pool(name="const", bufs=1) as const,
          tc.tile_pool(name="xpool", bufs=cfg.get("x_bufs", 3)) as xpool,
      ):
        for _rep in range(reps):
            # ---- resident tiles ----
            if use_fp8:
                wg8_sb = const.tile([P, KC // 2, 2, F], fp8, name="wg8_sb")
            else:
                wg_sb = [
                    const.tile([P, F], bf16, tag=f"wg{k}", name=f"wg_sb{k}")
                    for k in range(KC)
                ]
            bg_sb = const.tile([P, JC], f32)
            w1_sb = const.tile([P, V * KC, H1], bf16)
            w2_sb = const.tile([P, 2, OUT], bf16)
            b1_sb = const.tile([1, H1], f32)
            b2_sb = const.tile([1, OUT], f32)
            ones_sb = const.tile([1, BL], f32)
            id8_sb = const.tile([BL, BL], f32)
            pooledT = const.tile([P, JC, BL * V], f32)

            xt_tiles = {}

            if use_fp8:

                def load_x(g):
                    t = xpool.tile([P, KC // 2, 2, GRP], fp8, tag="x", name="xt")
                    src = xt_d[:, g * GRP : (g + 1) * GRP].rearrange(
                        "(kk i p) n -> p kk i n", p=P, i=2
                    )
                    nc.sync.dma_start(t[:], src)
                    xt_tiles[g] = t

                load_x(0)
                nc.sync.dma_start(
                    wg8_sb[:],
                    wg_d.rearrange("(kk i p) m -> p kk i m", p=P, i=2),
                )
                load_x(1)
            else:

                def load_x(g):
                    t = xpool.tile([P, KC, GRP], bf16, tag="x", name="xt")
                    src = xt_d[:, g * GRP : (g + 1) * GRP].rearrange(
                        "(k p) n -> p k n", p=P
                    )
                    nc.sync.dma_start(t[:], src)
                    xt_tiles[g] = t

                load_x(0)
                # W_gcn: separate tiles so the first Y-matmuls only wait on k=0
                for k in range(KC):
                    nc.sync.dma_start(wg_sb[k][:], wg_d[k * P : (k + 1) * P, :])
                load_x(1)
            nc.sync.dma_start(bg_sb[:], bg_d.rearrange("(j p) -> p j", p=P))
            nc.sync.dma_start(b1_sb[:], b1_d.rearrange("(o n) -> o n", o=1))
            nc.sync.dma_start(b2_sb[:], b2_d.rearrange("(o n) -> o n", o=1))
            nc.sync.dma_start(id8_sb[:], id8_d[:])
            nc.gpsimd.memset(ones_sb[:], 1.0)
            # W1/W2: 2MB bf16 loads straight into the resident tiles
            for gq in range(2):
                nc.sync.dma_start(
                    w1_sb[:, gq * 32 : (gq + 1) * 32, :],
                    w1_d[gq * 32 * P : (gq + 1) * 32 * P, :].rearrange(
                        "(i p) n -> p i n", p=P
                    ),
                )
            nc.sync.dma_start(
                w2_sb[:], w2_d[:].rearrange("(m p) n -> p m n", p=P)
            )

            # ---- main loop: GCN matmul + max-pool, one group ahead on DMA ----
            with tc.tile_pool(
                name=f"mpsum{_rep}", bufs=cfg.get("psum_bufs", 3), space="PSUM"
            ) as mpsum:
                for g in range(NG):
                    if g + 2 < NG:
                        load_x(g + 2)
                    xt = xt_tiles.pop(g)
                    if use_fp8:
                        for j in range(JC):
                            # 2-bank PSUM tile; per half-group accumulation
                            # chains share the stationary W block (kk, j)
                            yp = mpsum.tile([P, GRP], f32, tag="yp", name="yp")
                            for kk in range(KC // 2):
                                for h in range(2):
                                    nc.tensor.matmul(
                                        yp[
                                            :,
                                            h * (GRP // 2) : (h + 1) * (GRP // 2),
                                        ],
                                        wg8_sb[:, kk, :, j * P : (j + 1) * P],
                                        xt[
                                            :, kk, :,
                                            h * (GRP // 2) : (h + 1) * (GRP // 2),
                                        ],
                                        start=(kk == 0),
                                        stop=(kk == KC // 2 - 1),
                                        perf_mode=mybir.MatmulPerfMode.DoubleRow,
                                    )
                            nc.vector.reduce_max(
                                pooledT[
                                    :, j, g * (GRP // T) : (g + 1) * (GRP // T)
                                ],
                                yp[:].rearrange("p (q t) -> p q t", t=T),
                                axis=mybir.AxisListType.X,
                            )
                    else:
                        for h in range(2):
                            hh = g * 2 + h
                            for j in range(JC):
                                yp = mpsum.tile(
                                    [P, GRP // 2], f32, tag="yp", name="yp"
                                )
                                for k in range(KC):
                                    nc.tensor.matmul(
                                        yp[:],
                                        wg_sb[k][:, j * P : (j + 1) * P],
                                        xt[
                                            :, k,
                                            h * (GRP // 2) : (h + 1) * (GRP // 2),
                                        ],
                                        start=(k == 0),
                                        stop=(k == KC - 1),
                                    )
                                nc.vector.reduce_max(
                                    pooledT[
                                        :, j,
                                        hh * (GRP // 2 // T)
                                        : (hh + 1) * (GRP // 2 // T),
                                    ],
                                    yp[:].rearrange("p (q t) -> p q t", t=T),
                                    axis=mybir.AxisListType.X,
                                )

            # ---- epilogue: bias (fused into bf16 cast), MLP ----
            with tc.tile_pool(name=f"lpsum{_rep}", bufs=1, space="PSUM") as lpsum:
                pooled_m = const.tile([P, JC, BL * V], bf16)
                for j in range(JC):
                    nc.scalar.activation(
                        pooled_m[:, j, :],
                        pooledT[:, j, :],
                        AF.Identity,
                        bias=bg_sb[:, j : j + 1],
                    )

                hp = lpsum.tile([BL, H1], f32, tag="hp")
                for v in range(V):
                    for fc in range(KC):
                        i = v * KC + fc
                        lhs = pooled_m[:, fc, :].rearrange("p (b w) -> p w b", w=V)[
                            :, v, :
                        ]
                        nc.tensor.matmul(
                            hp[:], lhs, w1_sb[:, i, :], start=(i == 0), stop=False
                        )
                nc.tensor.matmul(hp[:], ones_sb[:], b1_sb[:], start=False, stop=True)
                h_sb = const.tile([BL, H1], f32)
                nc.scalar.activation(h_sb[:], hp[:], AF.Relu)

                ht_sb = const.tile([P, 2, BL], bf16)
                for m in range(2):
                    thp = lpsum.tile([P, BL], f32, tag="thp")
                    nc.tensor.transpose(
                        thp[:], h_sb[:, m * P : (m + 1) * P], id8_sb[:]
                    )
                    nc.vector.tensor_copy(ht_sb[:, m, :], thp[:])

                op = lpsum.tile([BL, OUT], f32, tag="op")
                for m in range(2):
                    nc.tensor.matmul(
                        op[:], ht_sb[:, m, :], w2_sb[:, m, :], start=(m == 0),
                        stop=False,
                    )
                nc.tensor.matmul(op[:], ones_sb[:], b2_sb[:], start=False, stop=True)
                o_sb = const.tile([BL, OUT], f32)
                nc.scalar.activation(o_sb[:], op[:], AF.Sigmoid)
                nc.sync.dma_start(
                    out_d[:] if reps == 1 else out_d[_rep], o_sb[:]
                )

    nc.compile()
    return nc


def _get_state(cfg=None):
    global _STATE
    if _STATE is None:
        _STATE = _build_nc(cfg or CFG)
    return _STATE


def make_in_maps(videos, W_gcn, b_gcn, W1, b1, W2, b2):
    videos = np.asarray(videos, dtype=np.float32)
    # frame-0 -> frame-1 GCN edge combine (elementwise, commutes with @W_gcn)
    vc = videos.copy()
    vc[:, :, 1, :] = 0.5 * videos[:, :, 1, :] + (1.0 / np.sqrt(2.0)) * videos[
        :, :, 0, :
    ]
    use_fp8 = CFG.get("gcn_fp8", False)
    g_np = ml_dtypes.float8_e4m3 if use_fp8 else BF16
    vcb = vc.astype(g_np)
    id8 = np.eye(BL, dtype=np.float32)
    common = {
        "W_gcn": np.asarray(W_gcn, dtype=np.float32).astype(g_np),
        "b_gcn": np.asarray(b_gcn, dtype=np.float32),
        "W1": np.asarray(W1, dtype=np.float32).astype(BF16),
        "b1": np.asarray(b1, dtype=np.float32),
        "W2": np.asarray(W2, dtype=np.float32).astype(BF16),
        "b2": np.asarray(b2, dtype=np.float32),
        "id8": id8,
    }
    in_maps = []
    for i in range(NCORES):
        m = dict(common)
        m["videosT"] = np.ascontiguousarray(
            vcb[i * BL : (i + 1) * BL].reshape(NLOC, F).T
        )
        in_maps.append(m)
    return in_maps


_RUNNER = None


def _make_runner(nc):
    """Cached multi-core PJRT runner (mirrors bass2jax.run_bass_via_pjrt but
    jits once so repeated calls don't re-trace)."""
    import jax
    import numpy as _np
    from jax.experimental.shard_map import shard_map
    from jax.sharding import Mesh, PartitionSpec
    from concourse import bass2jax, mybir

    bass2jax.install_neuronx_cc_hook()
    assert nc.dbg_addr is None
    partition_name = (
        nc.partition_id_tensor.name if nc.partition_id_tensor is not None else None
    )

    in_names, out_names, out_avals, zero_outs = [], [], [], []
    for alloc in nc.m.functions[0].allocations:
        if not isinstance(alloc, mybir.MemoryLocationSet):
            continue
        name = alloc.memorylocations[0].name
        if alloc.kind == "ExternalInput":
            if name != partition_name:
                in_names.append(name)
        elif alloc.kind == "ExternalOutput":
            out_names.append(name)
            shape = tuple(alloc.tensor_shape)
            dtype = mybir.dt.np(alloc.dtype)
            out_avals.append(jax.core.ShapedArray(shape, dtype))
            zero_outs.append(_np.zeros(shape, dtype))
    n_params = len(in_names)
    n_outs = len(out_avals)
    all_names = in_names + out_names
    if partition_name is not None:
        all_names = all_names + [partition_name]

    def _body(*args):
        operands = list(args)
        if partition_name is not None:
            operands.append(bass2jax.partition_id_tensor())
        outs = bass2jax._bass_exec_p.bind(
            *operands,
            out_avals=tuple(out_avals),
            in_names=tuple(all_names),
            out_names=tuple(out_names),
            lowering_input_output_aliases=(),
            sim_require_finite=True,
            sim_require_nnan=True,
            nc=nc,
        )
        return tuple(outs)

    devices = jax.devices()[:NCORES]
    mesh = Mesh(np.asarray(devices), ("core",))
    in_specs = (PartitionSpec("core"),) * (n_params + n_outs)
    out_specs = (PartitionSpec("core"),) * n_outs
    sharded = jax.jit(
        shard_map(
            _body, mesh=mesh, in_specs=in_specs, out_specs=out_specs, check_rep=False
        ),
        keep_unused=True,
    )

    def run(in_maps, device_inputs=None, materialize=True):
        if device_inputs is None:
            device_inputs = prep(in_maps)
        out_arrs = sharded(*device_inputs)
        jax.block_until_ready(out_arrs)
        if not materialize:
            # timing mode: stop at device completion; skip the D2H pull
            return out_arrs
        return [
            {
                name: _np.asarray(out_arrs[i]).reshape(NCORES, *out_avals[i].shape)[c]
                for i, name in enumerate(out_names)
            }
            for c in range(NCORES)
        ]

    def prep(in_maps):
        from jax.sharding import NamedSharding

        concat_in = [
            _np.concatenate([_np.asarray(in_maps[c][nm]) for c in range(NCORES)], 0)
            for nm in in_names
        ]
        concat_zeros = [
            _np.zeros((NCORES * z.shape[0], *z.shape[1:]), z.dtype) for z in zero_outs
        ]
        sh = NamedSharding(mesh, PartitionSpec("core"))
        arrs = [jax.device_put(a, sh) for a in concat_in + concat_zeros]
        jax.block_until_ready(arrs)
        return arrs

    return run, prep


def _get_runner():
    global _RUNNER
    if _RUNNER is None:
        _RUNNER = _make_runner(_get_state())
    return _RUNNER


def run_spmd(in_maps, device_inputs=None):
    run, _ = _get_runner()
    return run(in_maps, device_inputs)


def prep_inputs(in_maps):
    _, prep = _get_runner()
    return prep(in_maps)


def kernel(videos, W_gcn, b_gcn, W1, b1, W2, b2):
    in_maps = make_in_maps(videos, W_gcn, b_gcn, W1, b1, W2, b2)
    results = run_spmd(in_maps)
    out = np.stack([results[i]["out"] for i in range(NCORES)])  # [8, 8, 512]
    return out.reshape(B, OUT).reshape(B, V, T).astype(np.float32)


# revision 5
# speedup vs baseline: 1.2932x; 1.2932x over previous
"""Trainium2 Bass kernel for KeyframeSelectionNetwork (v2).

Math (per (b, v) video of T=64 frames, F=1024 features):
  GCN with self-loops + one edge (frame0 -> frame1), symmetric norm:
    out[t] = x[t] @ W_gcn                      for t != 1
    out[1] = (0.5*x[1] + (1/sqrt(2))*x[0]) @ W_gcn
  pooled = max_t out[t] + b_gcn
  h = relu(pooled.reshape(B, V*F) @ W1 + b1)
  key = sigmoid(h @ W2 + b2)            -> [B, V, T]

Strategy: data-parallel over batch across 8 cores. The frame-0/1 combine is
elementwise on X and commutes with the matmul, so it is applied on the HOST,
as is the [node, F] -> [F, node] transpose (pure layout) and the downcast
(X/W_gcn to fp8 e4m3, MLP weights to bf16; fp32 accumulation throughout
keeps rel-err ~9e-3, well under the 2e-2 gate). The device runs one clean
GEMM pipeline per core:
  - per 1024-node group g: load XT tile [128, kk, 2, 1024] fp8 (one 1MB DMA);
  - for each 128-wide output block j: PSUM-accumulated Double-FP8 matmuls
    (256-row contraction per instruction: W_gcn block [p,2,128] stationary,
    XT [p,2,512] moving) into a 2-bank [128,1024] PSUM tile;
  - one 3D-AP reduce_max over t per (g, j) writes pooledT [fout, b*v];
  - MLP in bf16: pooledT slices stationary vs W1 [128,256] moving, biases as
    rank-1 matmuls (ones.T @ b), relu/sigmoid on ACT, h transposed via PE.
"""

import sys

sys.path.insert(0, "/opt/trn_rl_repo")

import numpy as np
import ml_dtypes

BF16 = ml_dtypes.bfloat16

B, V, T, F = 64, 8, 64, 1024
NCORES = 8
BL = B // NCORES  # batches per core
NLOC = BL * V * T  # nodes per core (4096)
H1 = 256
OUT = V * T  # 512
P = 128
GRP = 1024  # nodes per group (matmul moving-N)
NG = NLOC // GRP  # 4 groups
KC = F // P  # 8 contraction chunks
JC = F // P  # 8 output-feature chunks

CFG = dict(
    psum_bufs=3,
    x_bufs=3,
    gcn_fp8=True,  # e4m3 X/W_gcn + DoubleRow matmul (2x PE rate, halved X bytes)
)

_STATE = None


def _build_nc(cfg, reps=1):
    import concourse.bacc as bacc
    import concourse.tile as tile
    from concourse import mybir

    f32 = mybir.dt.float32
    bf16 = mybir.dt.bfloat16
    fp8 = mybir.dt.float8e4
    AF = mybir.ActivationFunctionType
    use_fp8 = cfg.get("gcn_fp8", False)
    g_dt = fp8 if use_fp8 else bf16

    nc = bacc.Bacc(None, target_bir_lowering=False, debug=False)
    xt_d = nc.dram_tensor("videosT", [F, NLOC], g_dt, kind="ExternalInput")
    wg_d = nc.dram_tensor("W_gcn", [F, F], g_dt, kind="ExternalInput")
    bg_d = nc.dram_tensor("b_gcn", [F], f32, kind="ExternalInput")
    w1_d = nc.dram_tensor("W1", [V * F, H1], bf16, kind="ExternalInput")
    b1_d = nc.dram_tensor("b1", [H1], f32, kind="ExternalInput")
    w2_d = nc.dram_tensor("W2", [H1, OUT], bf16, kind="ExternalInput")
    b2_d = nc.dram_tensor("b2", [OUT], f32, kind="ExternalInput")
    id8_d = nc.dram_tensor("id8", [BL, BL], f32, kind="ExternalInput")
    if reps == 1:
        out_d = nc.dram_tensor("out", [BL, OUT], f32, kind="ExternalOutput")
    else:
        # distinct per-rep outputs so DCE can't drop repeated workloads
        out_d = nc.dram_tensor("out", [reps, BL, OUT], f32, kind="ExternalOutput")

    with tile.TileContext(nc) as tc:
      with (
          tc.tile_pool(name="const", bufs=1) as const,
          tc.tile_pool(name="xpool", bufs=cfg.get("x_bufs", 3)) as xpool,
      ):
        for _rep in range(reps):
            # ---- resident tiles ----
            if use_fp8:
                wg8_sb = const.tile([P, KC // 2, 2, F], fp8, name="wg8_sb")
            else:
                wg_sb = [
                    const.tile([P, F], bf16, tag=f"wg{k}", name=f"wg_sb{k}")
                    for k in range(KC)
                ]
            bg_sb = const.tile([P, JC], f32)
            w1_sb = const.tile([P, V * KC, H1], bf16)
            w2_sb = const.tile([P, 2, OUT], bf16)
            b1_sb = const.tile([1, H1], f32)
            b2_sb = const.tile([1, OUT], f32)
            ones_sb = const.tile([1, BL], f32)
            id8_sb = const.tile([BL, BL], f32)
            pooledT = const.tile([P, JC, BL * V], f32)

            xt_tiles = {}

            if use_fp8:

                def load_x(g):
                    t = xpool.tile([P, KC // 2, 2, GRP], fp8, tag="x", name="xt")
                    src = xt_d[:, g * GRP : (g + 1) * GRP].rearrange(
                        "(kk i p) n -> p kk i n", p=P, i=2
                    )
                    nc.sync.dma_start(t[:], src)
                    xt_tiles[g] = t

                load_x(0)
                wg_r = wg_d.rearrange("(kk i p) m -> p kk i m", p=P, i=2)
                nc.sync.dma_start(wg8_sb[:, 0:2], wg_r[:, 0:2])
                nc.sync.dma_start(wg8_sb[:, 2:4], wg_r[:, 2:4])
                load_x(1)
            else:

                def load_x(g):
                    t = xpool.tile([P, KC, GRP], bf16, tag="x", name="xt")
                    src = xt_d[:, g * GRP : (g + 1) * GRP].rearrange(
                        "(k p) n -> p k n", p=P
                    )
                    nc.sync.dma_start(t[:], src)
                    xt_tiles[g] = t

                load_x(0)
                # W_gcn: separate tiles so the first Y-matmuls only wait on k=0
                for k in range(KC):
                    nc.sync.dma_start(wg_sb[k][:], wg_d[k * P : (k + 1) * P, :])
                load_x(1)
            nc.sync.dma_start(bg_sb[:], bg_d.rearrange("(j p) -> p j", p=P))
            nc.sync.dma_start(b1_sb[:], b1_d.rearrange("(o n) -> o n", o=1))
            nc.sync.dma_start(b2_sb[:], b2_d.rearrange("(o n) -> o n", o=1))
            nc.sync.dma_start(id8_sb[:], id8_d[:])
            nc.gpsimd.memset(ones_sb[:], 1.0)
            # W1/W2: 2MB bf16 loads straight into the resident tiles
            for gq in range(2):
                nc.sync.dma_start(
                    w1_sb[:, gq * 32 : (gq + 1) * 32, :],
                    w1_d[gq * 32 * P : (gq + 1) * 32 * P, :].rearrange(
                        "(i p) n -> p i n", p=P
                    ),
                )
            nc.sync.dma_start(
                w2_sb[:], w2_d[:].rearrange("(m p) n -> p m n", p=P)
            )

            # ---- main loop: GCN matmul + max-pool, one group ahead on DMA ----
            with tc.tile_pool(
                name=f"mpsum{_rep}", bufs=cfg.get("psum_bufs", 3), space="PSUM"
            ) as mpsum:
                for g in range(NG):
                    if g + 2 < NG:
                        load_x(g + 2)
                    xt = xt_tiles.pop(g)
                    if use_fp8:
                        for j in range(JC):
                            # 2-bank PSUM tile; per half-group accumulation
                            # chains share the stationary W block (kk, j)
                            yp = mpsum.tile([P, GRP], f32, tag="yp", name="yp")
                            for kk in range(KC // 2):
                                for h in range(2):
                                    nc.tensor.matmul(
                                        yp[
                                            :,
                                            h * (GRP // 2) : (h + 1) * (GRP // 2),
                                        ],
                                        wg8_sb[:, kk, :, j * P : (j + 1) * P],
                                        xt[
                                            :, kk, :,
                                            h * (GRP // 2) : (h + 1) * (GRP // 2),
                                        ],
                                        start=(kk == 0),
                                        stop=(kk == KC // 2 - 1),
                                        perf_mode=mybir.MatmulPerfMode.DoubleRow,
                                    )
                            nc.vector.reduce_max(
                                pooledT[
                                    :, j, g * (GRP // T) : (g + 1) * (GRP // T)
                                ],
                                yp[:].rearrange("p (q t) -> p q t", t=T),
                                axis=mybir.AxisListType.X,
                            )
                    else:
                        for h in range(2):
                            hh = g * 2 + h
                            for j in range(JC):
                                yp = mpsum.tile(
                                    [P, GRP // 2], f32, tag="yp", name="yp"
                                )
                                for k in range(KC):
                                    nc.tensor.matmul(
                                        yp[:],
                                        wg_sb[k][:, j * P : (j + 1) * P],
                                        xt[
                                            :, k,
                                            h * (GRP // 2) : (h + 1) * (GRP // 2),
                                        ],
                                        start=(k == 0),
                                        stop=(k == KC - 1),
                                    )
                                nc.vector.reduce_max(
                                    pooledT[
                                        :, j,
                                        hh * (GRP // 2 // T)
                                        : (hh + 1) * (GRP // 2 // T),
                                    ],
                                    yp[:].rearrange("p (q t) -> p q t", t=T),
                                    axis=mybir.AxisListType.X,
                                )

            # ---- epilogue: bias (fused into bf16 cast), MLP ----
            with tc.tile_pool(name=f"lpsum{_rep}", bufs=1, space="PSUM") as lpsum:
                pooled_m = const.tile([P, JC, BL * V], bf16)
                hp = lpsum.tile([BL, H1], f32, tag="hp")
                for fc in range(KC):
                    nc.scalar.activation(
                        pooled_m[:, fc, :],
                        pooledT[:, fc, :],
                        AF.Identity,
                        bias=bg_sb[:, fc : fc + 1],
                    )
                    for v in range(V):
                        lhs = pooled_m[:, fc, :].rearrange("p (b w) -> p w b", w=V)[
                            :, v, :
                        ]
                        nc.tensor.matmul(
                            hp[:],
                            lhs,
                            w1_sb[:, v * KC + fc, :],
                            start=(fc == 0 and v == 0),
                            stop=False,
                        )
                nc.tensor.matmul(hp[:], ones_sb[:], b1_sb[:], start=False, stop=True)
                h_sb = const.tile([BL, H1], f32)
                nc.scalar.activation(h_sb[:], hp[:], AF.Relu)

                ht_sb = const.tile([P, 2, BL], bf16)
                for m in range(2):
                    thp = lpsum.tile([P, BL], f32, tag="thp")
                    nc.tensor.transpose(
                        thp[:], h_sb[:, m * P : (m + 1) * P], id8_sb[:]
                    )
                    nc.vector.tensor_copy(ht_sb[:, m, :], thp[:])

                op = lpsum.tile([BL, OUT], f32, tag="op")
                for m in range(2):
                    nc.tensor.matmul(
                        op[:], ht_sb[:, m, :], w2_sb[:, m, :], start=(m == 0),
                        stop=False,
                    )
                nc.tensor.matmul(op[:], ones_sb[:], b2_sb[:], start=False, stop=True)
                o_sb = const.tile([BL, OUT], f32)
                nc.scalar.activation(o_sb[:], op[:], AF.Sigmoid)
                nc.sync.dma_start(
                    out_d[:] if reps == 1 else out_d[_rep], o_sb[:]
                )

    nc.compile()
    return nc


def _get_state(cfg=None):
    global _STATE
    if _STATE is None:
        _STATE = _build_nc(cfg or CFG)
    return _STATE


def make_in_maps(videos, W_gcn, b_gcn, W1, b1, W2, b2):
    videos = np.asarray(videos, dtype=np.float32)
    # frame-0 -> frame-1 GCN edge combine (elementwise, commutes with @W_gcn)
    vc = videos.copy()
    vc[:, :, 1, :] = 0.5 * videos[:, :, 1, :] + (1.0 / np.sqrt(2.0)) * videos[
        :, :, 0, :
    ]
    use_fp8 = CFG.get("gcn_fp8", False)
    g_np = ml_dtypes.float8_e4m3 if use_fp8 else BF16
    vcb = vc.astype(g_np)
    id8 = np.eye(BL, dtype=np.float32)
    common = {
        "W_gcn": np.asarray(W_gcn, dtype=np.float32).astype(g_np),
        "b_gcn": np.asarray(b_gcn, dtype=np.float32),
        "W1": np.asarray(W1, dtype=np.float32).astype(BF16),
        "b1": np.asarray(b1, dtype=np.float32),
        "W2": np.asarray(W2, dtype=np.float32).astype(BF16),
        "b2": np.asarray(b2, dtype=np.float32),
        "id8": id8,
    }
    in_maps = []
    for i in range(NCORES):
        m = dict(common)
        m["videosT"] = np.ascontiguousarray(
            vcb[i * BL : (i + 1) * BL].reshape(NLOC, F).T
        )
        in_maps.append(m)
    return in_maps


_RUNNER = None


def _make_runner(nc):
    """Cached multi-core PJRT runner (mirrors bass2jax.run_bass_via_pjrt but
    jits once so repeated calls don't re-trace)."""
    import jax
    import numpy as _np
    from jax.experimental.shard_map import shard_map
    from jax.sharding import Mesh, PartitionSpec
    from concourse import bass2jax, mybir

    bass2jax.install_neuronx_cc_hook()
    assert nc.dbg_addr is None
    partition_name = (
        nc.partition_id_tensor.name if nc.partition_id_tensor is not None else None
    )

    in_names, out_names, out_avals, zero_outs = [], [], [], []
    for alloc in nc.m.functions[0].allocations:
        if not isinstance(alloc, mybir.MemoryLocationSet):
            continue
        name = alloc.memorylocations[0].name
        if alloc.kind == "ExternalInput":
            if name != partition_name:
                in_names.append(name)
        elif alloc.kind == "ExternalOutput":
            out_names.append(name)
            shape = tuple(alloc.tensor_shape)
            dtype = mybir.dt.np(alloc.dtype)
            out_avals.append(jax.core.ShapedArray(shape, dtype))
            zero_outs.append(_np.zeros(shape, dtype))
    n_params = len(in_names)
    n_outs = len(out_avals)
    all_names = in_names + out_names
    if partition_name is not None:
        all_names = all_names + [partition_name]

    def _body(*args):
        operands = list(args)
        if partition_name is not None:
            operands.append(bass2jax.partition_id_tensor())
        outs = bass2jax._bass_exec_p.bind(
            *operands,
            out_avals=tuple(out_avals),
            in_names=tuple(all_names),
            out_names=tuple(out_names),
            lowering_input_output_aliases=(),
            sim_require_finite=True,
            sim_require_nnan=True,
            nc=nc,
        )
        return tuple(outs)

    devices = jax.devices()[:NCORES]
    mesh = Mesh(np.asarray(devices), ("core",))
    in_specs = (PartitionSpec("core"),) * (n_params + n_outs)
    out_specs = (PartitionSpec("core"),) * n_outs
    sharded = jax.jit(
        shard_map(
            _body, mesh=mesh, in_specs=in_specs, out_specs=out_specs, check_rep=False
        ),
        keep_unused=True,
    )

    def run(in_maps, device_inputs=None, materialize=True):
        if device_inputs is None:
            device_inputs = prep(in_maps)
        out_arrs = sharded(*device_inputs)
        jax.block_until_ready(out_arrs)
        if not materialize:
            # timing mode: stop at device completion; skip the D2H pull
            return out_arrs
        return [
            {
                name: _np.asarray(out_arrs[i]).reshape(NCORES, *out_avals[i].shape)[c]
                for i, name in enumerate(out_names)
            }
            for c in range(NCORES)
        ]

    def prep(in_maps):
        from jax.sharding import NamedSharding

        concat_in = [
            _np.concatenate([_np.asarray(in_maps[c][nm]) for c in range(NCORES)], 0)
            for nm in in_names
        ]
        concat_zeros = [
            _np.zeros((NCORES * z.shape[0], *z.shape[1:]), z.dtype) for z in zero_outs
        ]
        sh = NamedSharding(mesh, PartitionSpec("core"))
        arrs = [jax.device_put(a, sh) for a in concat_in + concat_zeros]
        jax.block_until_ready(arrs)
        return arrs

    return run, prep


def _get_runner():
    global _RUNNER
    if _RUNNER is None:
        _RUNNER = _make_runner(_get_state())
    return _RUNNER


def run_spmd(in_maps, device_inputs=None):
    run, _ = _get_runner()
    return run(in_maps, device_inputs)


def prep_inputs(in_maps):
    _, prep = _get_runner()
    return prep(in_maps)


def kernel(videos, W_gcn, b_gcn, W1, b1, W2, b2):
    in_maps = make_in_maps(videos, W_gcn, b_gcn, W1, b1, W2, b2)
    results = run_spmd(in_maps)
    out = np.stack([results[i]["out"] for i in range(NCORES)])  # [8, 8, 512]
    return out.reshape(B, OUT).reshape(B, V, T).astype(np.float32)
